# revision 22
# baseline (speedup 1.0000x reference)
"""Trainium2 Bass kernel for nn_Attention_84585085927925 — Gram/M-path variant.

Reference (per batch element b, all fp32):
    qkv = x @ w_qkv.T ; q,k,v heads of 64 ; attn = sqrt(64) * q @ k.T (NO
    softmax) ; out = attn @ v ; out = out @ w_fc.T + b_fc

With no softmax the attention is linear in x, so the whole layer collapses
to out = x @ M + b_fc with a data-dependent [768,768] matrix M:
    C   = x.T x                       (symmetric: upper blocks + PE transpose)
    T1  = C @ wv.T                    [768, 768]
    G_h = s * wk_h @ C @ wv_h.T       per head (block-diag pairs, from T1)
    A   = per-pair G2T.T @ wfcT       [768, 768]
    M   = wq.T @ A                    [768, 768]
    out = x @ M + b_fc                computed as outT = M.T-stationary @ xT
One batch element per NeuronCore (8 cores, no collectives). All matmul
inputs fp16 (fp32 PSUM accumulation): ~7e-4 end-to-end max rel error.
All weights (+ identity for PE transpose, + fp16 bias) are packed into one
DRAM tensor loaded with 2 large DMAs: HWDGE descriptor generation costs
~630ns serialized per dma_start, so few large transfers win.
"""

import numpy as np

import concourse.bass as bass  # noqa: F401  (registers engine namespaces)
import concourse.mybir as mybir
import concourse.tile as tile
from concourse import bacc, bass_utils

F32 = mybir.dt.float32
F16 = mybir.dt.float16

B, N, D, H = 8, 1024, 768, 12
HD = D // H            # 64
SCALE = float(np.sqrt(HD))
P = 128
DT = D // P            # 6  d-tiles
NT = N // P            # 8  n(token)-tiles
NPAIR = H // 2         # 6 head pairs
# wcat row-block indices (each block is 128 rows of the packed tensor)
WV0, WK0, IDB, WFC0, WQ0 = 0, 6, 12, 13, 19
WCAT = 25


def _build_program():
    nc = bacc.Bacc(
        trn_type="TRN2", target_bir_lowering=False, debug=False, num_devices=B
    )
    xN_d = nc.dram_tensor("xN", [N, D], F16, kind="ExternalInput").ap()
    xT_d = nc.dram_tensor("xT", [D, N], F16, kind="ExternalInput").ap()
    wcat_d = nc.dram_tensor("wcat", [WCAT * P, D], F16,
                            kind="ExternalInput").ap()
    outT_d = nc.dram_tensor("outT", [D, N], F16, kind="ExternalOutput").ap()

    xN_r = xN_d.rearrange("(o p) d -> p o d", p=P)
    xT_r = xT_d.rearrange("(o p) n -> p o n", p=P)
    wcat_r = wcat_d.rearrange("(o p) c -> p o c", p=P)
    outT_r = outT_d.rearrange("(o p) n -> p o n", p=P)

    with tile.TileContext(nc) as tc:
        with tc.tile_pool(name="big", bufs=1) as big, \
             tc.tile_pool(name="outsp", bufs=6) as outsp, \
             tc.tile_pool(name="psp", bufs=6, space="PSUM") as psp, \
             tc.tile_pool(name="psg", bufs=2, space="PSUM") as psg:

            xN_sb = big.tile([P, NT, D], F16, name="xN_sb")
            xT_sb = big.tile([P, DT, N], F16, name="xT_sb")
            w_sb = big.tile([P, WCAT, D], F16, name="w_sb")
            c_sb = big.tile([P, DT, D], F16, name="c_sb")
            t1_sb = big.tile([P, DT, D], F16, name="t1_sb")
            g2t_sb = big.tile([P, NPAIR, P], F16, name="g2t_sb")
            a_sb = big.tile([P, NPAIR, D], F16, name="a_sb")
            m_sb = big.tile([P, DT, D], F16, name="m_sb")
            bias_sb = big.tile([P, DT], F32, name="bias_sb")

            id_ap = w_sb[:, IDB, 0:P]

            # ---- DMA loads: few, large transfers; xN first, split across
            # both HWDGE engines so C can start as soon as tiles land.
            jw = big.tile([P, 512], F16, name="jw")
            nc.vector.memset(jw[:], 0.0)
            nc.scalar.dma_start(xN_sb[:, 0, :], xN_r[:, 0, :])
            nc.sync.dma_start(xN_sb[:, 1:3, :], xN_r[:, 1:3, :])
            nc.scalar.dma_start(xN_sb[:, 3:5, :], xN_r[:, 3:5, :])
            nc.sync.dma_start(xN_sb[:, 5:8, :], xN_r[:, 5:8, :])
            nc.sync.dma_start(w_sb[:, 0:IDB + 1, :], wcat_r[:, 0:IDB + 1, :])
            nc.sync.dma_start(w_sb[:, IDB + 1:, :], wcat_r[:, IDB + 1:, :])
            nc.sync.dma_start(xT_sb[:], xT_r[:])
            nc.vector.memset(g2t_sb[:], 0.0)

            # ---- PE p-state warmup: data-independent matmuls on zeros so
            # the DVFS ramp burns during the initial DMA wait, not on real
            # work.  Results are never consumed.
            for _ in range(10):
                pw = psp.tile([P, 512], F32, tag="ps", name="pw")
                nc.tensor.matmul(pw[:, :256], jw[:, 0:128], jw[:, :256],
                                 start=True, stop=True)

            copy_engines = [nc.vector.tensor_copy, nc.scalar.copy]
            cp_i = 0

            def copy(dst, src):
                nonlocal cp_i
                copy_engines[cp_i % 2](dst, src)
                cp_i += 1

            # ---- C = x.T x, upper-triangular 128-blocks, nt-outer groups ----
            # row-tile a covers cols [128a, 768) in chunks <= 384 wide
            groups = [
                [(0, 0, 384), (0, 384, 384), (1, 128, 384), (1, 512, 256)],
                [(2, 256, 384), (2, 640, 128), (3, 384, 384)],
                [(4, 512, 256), (5, 640, 128)],
            ]
            for grp in groups:
                tiles = [psp.tile([P, 512], F32, tag="ps", name="pc")
                         for _ in grp]
                for nt in range(NT):
                    for (a, c0, w), pt in zip(grp, tiles):
                        nc.tensor.matmul(
                            pt[:, :w],
                            xN_sb[:, nt, a * P:(a + 1) * P],
                            xN_sb[:, nt, c0:c0 + w],
                            start=(nt == 0), stop=(nt == NT - 1),
                        )
                for (a, c0, w), pt in zip(grp, tiles):
                    if a >= 4:
                        # last group feeds the first transposes immediately;
                        # keep these on the (fast, unloaded) vector engine
                        nc.vector.tensor_copy(c_sb[:, a, c0:c0 + w],
                                              pt[:, :w])
                    else:
                        copy(c_sb[:, a, c0:c0 + w], pt[:, :w])

            # ---- T1 = C @ wvT rows desc; PE-transpose lower C blocks ----
            # row a needs lhsT blocks (d2, a): for d2 > a transpose stored
            # (a, d2).  Emit transposes two rows ahead of their T1 use.
            def emit_transposes(a):
                for b in range(a + 1, DT):
                    tp = psg.tile([P, P], F16, tag="ptr", bufs=2, name="tp")
                    nc.tensor.transpose(tp[:], c_sb[:, a, b * P:(b + 1) * P],
                                        id_ap)
                    nc.vector.tensor_copy(c_sb[:, b, a * P:(a + 1) * P], tp[:])

            emit_transposes(4)
            for a in [5, 4, 3, 2, 1, 0]:
                if a >= 2:
                    emit_transposes(a - 2)
                for ch in range(2):
                    pt = psp.tile([P, 512], F32, tag="ps", name="pt1")
                    for d2t in range(DT):
                        nc.tensor.matmul(
                            pt[:, :384],
                            c_sb[:, d2t, a * P:(a + 1) * P],
                            w_sb[:, WV0 + d2t, ch * 384:(ch + 1) * 384],
                            start=(d2t == 0), stop=(d2t == DT - 1),
                        )
                    copy(t1_sb[:, a, ch * 384:(ch + 1) * 384], pt[:, :384])

            # ---- G2T per pair: [vf, kf] = sum_d T1[d, vf] wkT8[d, kf] ----
            for t in range(NPAIR):
                pg = psp.tile([P, 512], F32, tag="ps", name="pg")
                for dt in range(DT):
                    nc.tensor.matmul(
                        pg[:, :P],
                        t1_sb[:, dt, t * P:(t + 1) * P],
                        w_sb[:, WK0 + dt, t * P:(t + 1) * P],
                        start=(dt == 0), stop=(dt == DT - 1),
                    )
                nc.vector.tensor_copy(g2t_sb[0:64, t, 0:64], pg[0:64, 0:64])
                nc.scalar.copy(g2t_sb[64:128, t, 64:128], pg[64:128, 64:128])

            # ---- A[kf, e] = sum_vf G2T[vf, kf] wfcT[vf, e] per pair ----
            for t in range(NPAIR):
                for ch in range(2):
                    pa = psp.tile([P, 512], F32, tag="ps", name="pa")
                    nc.tensor.matmul(
                        pa[:, :384],
                        g2t_sb[:, t, :],
                        w_sb[:, WFC0 + t, ch * 384:(ch + 1) * 384],
                        start=True, stop=True,
                    )
                    copy(a_sb[:, t, ch * 384:(ch + 1) * 384], pa[:, :384])

            # ---- M[d, e] = sum_kf wq[kf, d] A[kf, e] ----
            for dtile in range(DT):
                for ch in range(2):
                    pm = psp.tile([P, 512], F32, tag="ps", name="pm")
                    for kft in range(DT):
                        nc.tensor.matmul(
                            pm[:, :384],
                            w_sb[:, WQ0 + kft, dtile * P:(dtile + 1) * P],
                            a_sb[:, kft, ch * 384:(ch + 1) * 384],
                            start=(kft == 0), stop=(kft == DT - 1),
                        )
                    copy(m_sb[:, dtile, ch * 384:(ch + 1) * 384], pm[:, :384])

            # ---- outT[e, n] = sum_d M[d, e] xT[d, n] + b[e] ----
            # bias scalar operand for tensor_scalar_add must be f32
            nc.scalar.copy(bias_sb[:], w_sb[:, IDB, P:P + DT])
            for et in range(DT):
                ot = outsp.tile([P, N], F16, tag="ot", name="ot")
                bias_ap = bias_sb[:, et:et + 1]
                for nch in range(2):
                    po = psp.tile([P, 512], F32, tag="ps", name="po")
                    for dt in range(DT):
                        nc.tensor.matmul(
                            po[:],
                            m_sb[:, dt, et * P:(et + 1) * P],
                            xT_sb[:, dt, nch * 512:(nch + 1) * 512],
                            start=(dt == 0), stop=(dt == DT - 1),
                        )
                    dst = ot[:, nch * 512:(nch + 1) * 512]
                    nc.vector.tensor_scalar_add(dst, po[:], bias_ap)
                    if et == DT - 1:
                        # last row block: store each half as soon as its
                        # bias-add lands to shorten the tail
                        nc.scalar.dma_start(
                            outT_r[:, et, nch * 512:(nch + 1) * 512], dst)
                if et < DT - 1:
                    # store via the Activation-engine HWDGE (its own
                    # descriptor generator; sync's is busy with loads)
                    nc.scalar.dma_start(outT_r[:, et, :], ot[:])

    nc.compile()
    return nc


_NC_CACHE = None
LAST_EXEC_NS = None
LAST_RES = None


def kernel(x, w_qkv, w_fc, b_fc, _trace=False):
    global _NC_CACHE, LAST_EXEC_NS, LAST_RES
    x = np.asarray(x, dtype=np.float32)
    w_qkv = np.asarray(w_qkv, dtype=np.float32)
    w_fc = np.asarray(w_fc, dtype=np.float32)
    b_fc = np.asarray(b_fc, dtype=np.float32)

    if _NC_CACHE is None:
        _NC_CACHE = _build_program()
    nc = _NC_CACHE

    f16 = np.float16
    wcat = np.zeros((WCAT * P, D), dtype=f16)
    wcat[WV0 * P:(WV0 + 6) * P] = w_qkv[2 * D:].T.astype(f16)          # wvT
    wcat[WK0 * P:(WK0 + 6) * P] = (SCALE * w_qkv[D:2 * D]).T.astype(f16)
    wcat[WFC0 * P:(WFC0 + 6) * P] = w_fc.T.astype(f16)                 # wfcT
    wcat[WQ0 * P:(WQ0 + 6) * P] = w_qkv[:D].astype(f16)                # wqN
    idb = wcat[IDB * P:(IDB + 1) * P]
    idb[:, 0:P] = np.eye(P, dtype=f16)
    idb[:, P:P + DT] = b_fc.astype(f16).reshape(DT, P).T               # bias

    in_maps = []
    for b in range(B):
        in_maps.append({
            "xN": x[b].astype(f16),
            "xT": np.ascontiguousarray(x[b].T).astype(f16),
            "wcat": wcat,
        })

    res = bass_utils.run_bass_kernel_spmd(
        nc, in_maps, core_ids=list(range(B)), trace=_trace
    )
    LAST_EXEC_NS = res.exec_time_ns
    LAST_RES = res
    out = np.stack([res.results[b]["outT"].T.astype(np.float32)
                    for b in range(B)])
    return np.ascontiguousarray(out)


# revision 23
# speedup vs baseline: 1.0555x; 1.0555x over previous
"""Trainium2 Bass kernel for nn_Attention_84585085927925 — Gram/M-path variant.

Reference (per batch element b, all fp32):
    qkv = x @ w_qkv.T ; q,k,v heads of 64 ; attn = sqrt(64) * q @ k.T (NO
    softmax) ; out = attn @ v ; out = out @ w_fc.T + b_fc

With no softmax the attention is linear in x, so the whole layer collapses
to out = x @ M + b_fc with a data-dependent [768,768] matrix M:
    C   = x.T x                       (symmetric: upper blocks + PE transpose)
    T1  = C @ wv.T                    [768, 768]
    G_h = s * wk_h @ C @ wv_h.T       per head (block-diag pairs, from T1)
    A   = per-pair G2T.T @ wfcT       [768, 768]
    M   = wq.T @ A                    [768, 768]
    out = x @ M + b_fc                computed as outT = M.T-stationary @ xT
One batch element per NeuronCore (8 cores, no collectives). All matmul
inputs fp16 (fp32 PSUM accumulation): ~7e-4 end-to-end max rel error.
All weights (+ identity for PE transpose, + fp16 bias) are packed into one
DRAM tensor loaded with 2 large DMAs: HWDGE descriptor generation costs
~630ns serialized per dma_start, so few large transfers win.
"""

import numpy as np

import concourse.bass as bass  # noqa: F401  (registers engine namespaces)
import concourse.mybir as mybir
import concourse.tile as tile
from concourse import bacc, bass_utils

F32 = mybir.dt.float32
F16 = mybir.dt.float16

B, N, D, H = 8, 1024, 768, 12
HD = D // H            # 64
SCALE = float(np.sqrt(HD))
P = 128
DT = D // P            # 6  d-tiles
NT = N // P            # 8  n(token)-tiles
NPAIR = H // 2         # 6 head pairs
# wcat row-block indices (each block is 128 rows of the packed tensor)
WV0, WK0, IDB, WFC0, WQ0 = 0, 6, 12, 13, 19
WCAT = 25


def _build_program():
    nc = bacc.Bacc(
        trn_type="TRN2", target_bir_lowering=False, debug=False, num_devices=B
    )
    xN_d = nc.dram_tensor("xN", [N, D], F16, kind="ExternalInput").ap()
    xT_d = nc.dram_tensor("xT", [D, N], F16, kind="ExternalInput").ap()
    wcat_d = nc.dram_tensor("wcat", [WCAT * P, D], F16,
                            kind="ExternalInput").ap()
    outT_d = nc.dram_tensor("outT", [D, N], F16, kind="ExternalOutput").ap()

    xN_r = xN_d.rearrange("(o p) d -> p o d", p=P)
    xT_r = xT_d.rearrange("(o p) n -> p o n", p=P)
    wcat_r = wcat_d.rearrange("(o p) c -> p o c", p=P)
    outT_r = outT_d.rearrange("(o p) n -> p o n", p=P)

    with tile.TileContext(nc) as tc:
        with tc.tile_pool(name="big", bufs=1) as big, \
             tc.tile_pool(name="outsp", bufs=6) as outsp, \
             tc.tile_pool(name="psp", bufs=6, space="PSUM") as psp, \
             tc.tile_pool(name="psg", bufs=2, space="PSUM") as psg:

            xN_sb = big.tile([P, NT, D], F16, name="xN_sb")
            xT_sb = big.tile([P, DT, N], F16, name="xT_sb")
            w_sb = big.tile([P, WCAT, D], F16, name="w_sb")
            c_sb = big.tile([P, DT, D], F16, name="c_sb")
            t1_sb = big.tile([P, DT, D], F16, name="t1_sb")
            g2t_sb = big.tile([P, NPAIR, P], F16, name="g2t_sb")
            a_sb = big.tile([P, NPAIR, D], F16, name="a_sb")
            m_sb = big.tile([P, DT, D], F16, name="m_sb")
            bias_sb = big.tile([P, DT], F32, name="bias_sb")

            id_ap = w_sb[:, IDB, 0:P]

            # ---- DMA loads: few, large transfers; xN first, split across
            # both HWDGE engines so C can start as soon as tiles land.
            jw = big.tile([P, 512], F16, name="jw")
            nc.vector.memset(jw[:], 0.0)
            nc.scalar.dma_start(xN_sb[:, 0, :], xN_r[:, 0, :])
            nc.sync.dma_start(xN_sb[:, 1:3, :], xN_r[:, 1:3, :])
            nc.scalar.dma_start(xN_sb[:, 3:5, :], xN_r[:, 3:5, :])
            nc.sync.dma_start(xN_sb[:, 5:8, :], xN_r[:, 5:8, :])
            nc.sync.dma_start(w_sb[:, 0:IDB + 1, :], wcat_r[:, 0:IDB + 1, :])
            nc.sync.dma_start(w_sb[:, IDB + 1:, :], wcat_r[:, IDB + 1:, :])
            nc.sync.dma_start(xT_sb[:], xT_r[:])
            nc.vector.memset(g2t_sb[:], 0.0)

            # ---- PE p-state warmup: data-independent matmuls on zeros so
            # the DVFS ramp burns during the initial DMA wait, not on real
            # work.  Results are never consumed.
            for _ in range(10):
                pw = psp.tile([P, 512], F32, tag="ps", name="pw")
                nc.tensor.matmul(pw[:, :256], jw[:, 0:128], jw[:, :256],
                                 start=True, stop=True)

            copy_engines = [nc.vector.tensor_copy, nc.scalar.copy]
            cp_i = 0

            def copy(dst, src):
                nonlocal cp_i
                copy_engines[cp_i % 2](dst, src)
                cp_i += 1

            # ---- C = x.T x, upper-triangular 128-blocks, nt-outer passes ----
            # row-tile a covers cols [128a, 768) in chunks <= 384 wide.
            # Pass 1 (6 psum tiles) burns ~2us of compute per n-tile, so DMA
            # arrival of later x tiles stays ahead of consumption.
            groups = [
                [(0, 0, 384), (0, 384, 384), (1, 128, 384), (1, 512, 256),
                 (2, 256, 384), (2, 640, 128)],
                [(3, 384, 384), (4, 512, 256), (5, 640, 128)],
            ]
            for grp in groups:
                tiles = [psp.tile([P, 512], F32, tag="ps", name="pc")
                         for _ in grp]
                for nt in range(NT):
                    for (a, c0, w), pt in zip(grp, tiles):
                        nc.tensor.matmul(
                            pt[:, :w],
                            xN_sb[:, nt, a * P:(a + 1) * P],
                            xN_sb[:, nt, c0:c0 + w],
                            start=(nt == 0), stop=(nt == NT - 1),
                        )
                for (a, c0, w), pt in zip(grp, tiles):
                    if a >= 4:
                        # these feed the first transposes immediately; keep
                        # them on the (fast, unloaded) vector engine
                        nc.vector.tensor_copy(c_sb[:, a, c0:c0 + w],
                                              pt[:, :w])
                    else:
                        copy(c_sb[:, a, c0:c0 + w], pt[:, :w])

            # ---- T1 = C @ wvT rows desc; PE-transpose lower C blocks ----
            # row a needs lhsT blocks (d2, a): for d2 > a transpose stored
            # (a, d2).  Emit transposes two rows ahead of their T1 use.
            def emit_transposes(a):
                for b in range(a + 1, DT):
                    tp = psg.tile([P, P], F16, tag="ptr", bufs=2, name="tp")
                    nc.tensor.transpose(tp[:], c_sb[:, a, b * P:(b + 1) * P],
                                        id_ap)
                    nc.vector.tensor_copy(c_sb[:, b, a * P:(a + 1) * P], tp[:])

            emit_transposes(4)
            for a in [5, 4, 3, 2, 1, 0]:
                if a >= 2:
                    emit_transposes(a - 2)
                for ch in range(2):
                    pt = psp.tile([P, 512], F32, tag="ps", name="pt1")
                    for d2t in range(DT):
                        nc.tensor.matmul(
                            pt[:, :384],
                            c_sb[:, d2t, a * P:(a + 1) * P],
                            w_sb[:, WV0 + d2t, ch * 384:(ch + 1) * 384],
                            start=(d2t == 0), stop=(d2t == DT - 1),
                        )
                    copy(t1_sb[:, a, ch * 384:(ch + 1) * 384], pt[:, :384])

            # ---- G2T per pair: [vf, kf] = sum_d T1[d, vf] wkT8[d, kf] ----
            for t in range(NPAIR):
                pg = psp.tile([P, 512], F32, tag="ps", name="pg")
                for dt in range(DT):
                    nc.tensor.matmul(
                        pg[:, :P],
                        t1_sb[:, dt, t * P:(t + 1) * P],
                        w_sb[:, WK0 + dt, t * P:(t + 1) * P],
                        start=(dt == 0), stop=(dt == DT - 1),
                    )
                nc.vector.tensor_copy(g2t_sb[0:64, t, 0:64], pg[0:64, 0:64])
                nc.scalar.copy(g2t_sb[64:128, t, 64:128], pg[64:128, 64:128])

            # ---- A[kf, e] = sum_vf G2T[vf, kf] wfcT[vf, e] per pair ----
            for t in range(NPAIR):
                for ch in range(2):
                    pa = psp.tile([P, 512], F32, tag="ps", name="pa")
                    nc.tensor.matmul(
                        pa[:, :384],
                        g2t_sb[:, t, :],
                        w_sb[:, WFC0 + t, ch * 384:(ch + 1) * 384],
                        start=True, stop=True,
                    )
                    copy(a_sb[:, t, ch * 384:(ch + 1) * 384], pa[:, :384])

            # ---- M[d, e] = sum_kf wq[kf, d] A[kf, e] ----
            for dtile in range(DT):
                for ch in range(2):
                    pm = psp.tile([P, 512], F32, tag="ps", name="pm")
                    for kft in range(DT):
                        nc.tensor.matmul(
                            pm[:, :384],
                            w_sb[:, WQ0 + kft, dtile * P:(dtile + 1) * P],
                            a_sb[:, kft, ch * 384:(ch + 1) * 384],
                            start=(kft == 0), stop=(kft == DT - 1),
                        )
                    copy(m_sb[:, dtile, ch * 384:(ch + 1) * 384], pm[:, :384])

            # ---- outT[e, n] = sum_d M[d, e] xT[d, n] + b[e] ----
            # bias scalar operand for tensor_scalar_add must be f32
            nc.scalar.copy(bias_sb[:], w_sb[:, IDB, P:P + DT])
            for et in range(DT):
                ot = outsp.tile([P, N], F16, tag="ot", name="ot")
                bias_ap = bias_sb[:, et:et + 1]
                for nch in range(2):
                    po = psp.tile([P, 512], F32, tag="ps", name="po")
                    for dt in range(DT):
                        nc.tensor.matmul(
                            po[:],
                            m_sb[:, dt, et * P:(et + 1) * P],
                            xT_sb[:, dt, nch * 512:(nch + 1) * 512],
                            start=(dt == 0), stop=(dt == DT - 1),
                        )
                    dst = ot[:, nch * 512:(nch + 1) * 512]
                    nc.vector.tensor_scalar_add(dst, po[:], bias_ap)
                    if et == DT - 1:
                        # last row block: store each half as soon as its
                        # bias-add lands to shorten the tail
                        nc.scalar.dma_start(
                            outT_r[:, et, nch * 512:(nch + 1) * 512], dst)
                if et < DT - 1:
                    # store via the Activation-engine HWDGE (its own
                    # descriptor generator; sync's is busy with loads)
                    nc.scalar.dma_start(outT_r[:, et, :], ot[:])

    nc.compile()
    return nc


_NC_CACHE = None
LAST_EXEC_NS = None
LAST_RES = None


def kernel(x, w_qkv, w_fc, b_fc, _trace=False):
    global _NC_CACHE, LAST_EXEC_NS, LAST_RES
    x = np.asarray(x, dtype=np.float32)
    w_qkv = np.asarray(w_qkv, dtype=np.float32)
    w_fc = np.asarray(w_fc, dtype=np.float32)
    b_fc = np.asarray(b_fc, dtype=np.float32)

    if _NC_CACHE is None:
        _NC_CACHE = _build_program()
    nc = _NC_CACHE

    f16 = np.float16
    wcat = np.zeros((WCAT * P, D), dtype=f16)
    wcat[WV0 * P:(WV0 + 6) * P] = w_qkv[2 * D:].T.astype(f16)          # wvT
    wcat[WK0 * P:(WK0 + 6) * P] = (SCALE * w_qkv[D:2 * D]).T.astype(f16)
    wcat[WFC0 * P:(WFC0 + 6) * P] = w_fc.T.astype(f16)                 # wfcT
    wcat[WQ0 * P:(WQ0 + 6) * P] = w_qkv[:D].astype(f16)                # wqN
    idb = wcat[IDB * P:(IDB + 1) * P]
    idb[:, 0:P] = np.eye(P, dtype=f16)
    idb[:, P:P + DT] = b_fc.astype(f16).reshape(DT, P).T               # bias

    in_maps = []
    for b in range(B):
        in_maps.append({
            "xN": x[b].astype(f16),
            "xT": np.ascontiguousarray(x[b].T).astype(f16),
            "wcat": wcat,
        })

    res = bass_utils.run_bass_kernel_spmd(
        nc, in_maps, core_ids=list(range(B)), trace=_trace
    )
    LAST_EXEC_NS = res.exec_time_ns
    LAST_RES = res
    out = np.stack([res.results[b]["outT"].T.astype(np.float32)
                    for b in range(B)])
    return np.ascontiguousarray(out)


# revision 25
# speedup vs baseline: 1.0582x; 1.0026x over previous
"""Trainium2 Bass kernel for nn_Attention_84585085927925 — Gram/M-path variant.

Reference (per batch element b, all fp32):
    qkv = x @ w_qkv.T ; q,k,v heads of 64 ; attn = sqrt(64) * q @ k.T (NO
    softmax) ; out = attn @ v ; out = out @ w_fc.T + b_fc

With no softmax the attention is linear in x, so the whole layer collapses
to out = x @ M + b_fc with a data-dependent [768,768] matrix M:
    C   = x.T x                       (symmetric: upper blocks + PE transpose)
    T1  = C @ wv.T                    [768, 768]
    G_h = s * wk_h @ C @ wv_h.T       per head (block-diag pairs, from T1)
    A   = per-pair G2T.T @ wfcT       [768, 768]
    M   = wq.T @ A                    [768, 768]
    out = x @ M + b_fc                computed as outT = M.T-stationary @ xT
One batch element per NeuronCore (8 cores, no collectives). All matmul
inputs fp16 (fp32 PSUM accumulation): ~7e-4 end-to-end max rel error.
All weights (+ identity for PE transpose, + fp16 bias) are packed into one
DRAM tensor loaded with 2 large DMAs: HWDGE descriptor generation costs
~630ns serialized per dma_start, so few large transfers win.
"""

import numpy as np

import concourse.bass as bass  # noqa: F401  (registers engine namespaces)
import concourse.mybir as mybir
import concourse.tile as tile
from concourse import bacc, bass_utils

F32 = mybir.dt.float32
F16 = mybir.dt.float16

B, N, D, H = 8, 1024, 768, 12
HD = D // H            # 64
SCALE = float(np.sqrt(HD))
P = 128
DT = D // P            # 6  d-tiles
NT = N // P            # 8  n(token)-tiles
NPAIR = H // 2         # 6 head pairs
# wcat row-block indices (each block is 128 rows of the packed tensor)
WV0, WK0, IDB, WFC0, WQ0 = 0, 6, 12, 13, 19
WCAT = 25


def _build_program():
    nc = bacc.Bacc(
        trn_type="TRN2", target_bir_lowering=False, debug=False, num_devices=B
    )
    xN_d = nc.dram_tensor("xN", [N, D], F16, kind="ExternalInput").ap()
    xT_d = nc.dram_tensor("xT", [D, N], F16, kind="ExternalInput").ap()
    wcat_d = nc.dram_tensor("wcat", [WCAT * P, D], F16,
                            kind="ExternalInput").ap()
    outT_d = nc.dram_tensor("outT", [D, N], F16, kind="ExternalOutput").ap()

    xN_r = xN_d.rearrange("(o p) d -> p o d", p=P)
    xT_r = xT_d.rearrange("(o p) n -> p o n", p=P)
    wcat_r = wcat_d.rearrange("(o p) c -> p o c", p=P)
    outT_r = outT_d.rearrange("(o p) n -> p o n", p=P)

    with tile.TileContext(nc) as tc:
        with tc.tile_pool(name="big", bufs=1) as big, \
             tc.tile_pool(name="outsp", bufs=6) as outsp, \
             tc.tile_pool(name="psp", bufs=6, space="PSUM") as psp, \
             tc.tile_pool(name="psg", bufs=2, space="PSUM") as psg:

            xN_sb = big.tile([P, NT, D], F16, name="xN_sb")
            xT_sb = big.tile([P, DT, N], F16, name="xT_sb")
            w_sb = big.tile([P, WCAT, D], F16, name="w_sb")
            c_sb = big.tile([P, DT, D], F16, name="c_sb")
            t1_sb = big.tile([P, DT, D], F16, name="t1_sb")
            g2t_sb = big.tile([P, NPAIR, P], F16, name="g2t_sb")
            a_sb = big.tile([P, NPAIR, D], F16, name="a_sb")
            m_sb = big.tile([P, DT, D], F16, name="m_sb")
            bias_sb = big.tile([P, DT], F32, name="bias_sb")

            id_ap = w_sb[:, IDB, 0:P]

            # ---- DMA loads: few, large transfers; xN first, split across
            # both HWDGE engines so C can start as soon as tiles land.
            jw = big.tile([P, 512], F16, name="jw")
            nc.vector.memset(jw[:], 0.0)
            nc.scalar.dma_start(xN_sb[:, 0, :], xN_r[:, 0, :])
            nc.sync.dma_start(xN_sb[:, 1:3, :], xN_r[:, 1:3, :])
            nc.scalar.dma_start(xN_sb[:, 3:5, :], xN_r[:, 3:5, :])
            nc.sync.dma_start(xN_sb[:, 5:8, :], xN_r[:, 5:8, :])
            nc.sync.dma_start(w_sb[:, 0:IDB + 1, :], wcat_r[:, 0:IDB + 1, :])
            nc.sync.dma_start(w_sb[:, IDB + 1:, :], wcat_r[:, IDB + 1:, :])
            nc.sync.dma_start(xT_sb[:], xT_r[:])
            nc.vector.memset(g2t_sb[:], 0.0)

            # ---- PE p-state warmup: data-independent matmuls on zeros so
            # the DVFS ramp burns during the initial DMA wait, not on real
            # work.  Results are never consumed.
            for _ in range(12):
                pw = psp.tile([P, 512], F32, tag="ps", name="pw")
                nc.tensor.matmul(pw[:, :256], jw[:, 0:128], jw[:, :256],
                                 start=True, stop=True)

            copy_engines = [nc.vector.tensor_copy, nc.scalar.copy]
            cp_i = 0

            def copy(dst, src):
                nonlocal cp_i
                copy_engines[cp_i % 2](dst, src)
                cp_i += 1

            # ---- C = x.T x, upper-triangular 128-blocks, nt-outer passes ----
            # row-tile a covers cols [128a, 768) in chunks <= 384 wide.
            # Pass 1 (6 psum tiles) burns ~2us of compute per n-tile, so DMA
            # arrival of later x tiles stays ahead of consumption.
            groups = [
                [(0, 0, 384), (0, 384, 384), (1, 128, 384), (1, 512, 256),
                 (2, 256, 384), (2, 640, 128)],
                [(3, 384, 384), (4, 512, 256), (5, 640, 128)],
            ]
            for grp in groups:
                tiles = [psp.tile([P, 512], F32, tag="ps", name="pc")
                         for _ in grp]
                for nt in range(NT):
                    for (a, c0, w), pt in zip(grp, tiles):
                        nc.tensor.matmul(
                            pt[:, :w],
                            xN_sb[:, nt, a * P:(a + 1) * P],
                            xN_sb[:, nt, c0:c0 + w],
                            start=(nt == 0), stop=(nt == NT - 1),
                        )
                for (a, c0, w), pt in zip(grp, tiles):
                    if a >= 4:
                        # these feed the first transposes immediately; keep
                        # them on the (fast, unloaded) vector engine
                        nc.vector.tensor_copy(c_sb[:, a, c0:c0 + w],
                                              pt[:, :w])
                    else:
                        copy(c_sb[:, a, c0:c0 + w], pt[:, :w])

            # ---- T1 = C @ wvT rows desc; PE-transpose lower C blocks ----
            # row a needs lhsT blocks (d2, a): for d2 > a transpose stored
            # (a, d2).  Emit transposes two rows ahead of their T1 use.
            def emit_transposes(a):
                for b in range(a + 1, DT):
                    tp = psg.tile([P, P], F16, tag="ptr", bufs=2, name="tp")
                    nc.tensor.transpose(tp[:], c_sb[:, a, b * P:(b + 1) * P],
                                        id_ap)
                    nc.vector.tensor_copy(c_sb[:, b, a * P:(a + 1) * P], tp[:])

            emit_transposes(4)
            for a in [5, 4, 3, 2, 1, 0]:
                if a >= 2:
                    emit_transposes(a - 2)
                for ch in range(2):
                    pt = psp.tile([P, 512], F32, tag="ps", name="pt1")
                    for d2t in range(DT):
                        nc.tensor.matmul(
                            pt[:, :384],
                            c_sb[:, d2t, a * P:(a + 1) * P],
                            w_sb[:, WV0 + d2t, ch * 384:(ch + 1) * 384],
                            start=(d2t == 0), stop=(d2t == DT - 1),
                        )
                    copy(t1_sb[:, a, ch * 384:(ch + 1) * 384], pt[:, :384])

            # ---- G2T per pair: [vf, kf] = sum_d T1[d, vf] wkT8[d, kf] ----
            for t in range(NPAIR):
                pg = psp.tile([P, 512], F32, tag="ps", name="pg")
                for dt in range(DT):
                    nc.tensor.matmul(
                        pg[:, :P],
                        t1_sb[:, dt, t * P:(t + 1) * P],
                        w_sb[:, WK0 + dt, t * P:(t + 1) * P],
                        start=(dt == 0), stop=(dt == DT - 1),
                    )
                nc.vector.tensor_copy(g2t_sb[0:64, t, 0:64], pg[0:64, 0:64])
                nc.scalar.copy(g2t_sb[64:128, t, 64:128], pg[64:128, 64:128])

            # ---- A[kf, e] = sum_vf G2T[vf, kf] wfcT[vf, e] per pair ----
            for t in range(NPAIR):
                for ch in range(2):
                    pa = psp.tile([P, 512], F32, tag="ps", name="pa")
                    nc.tensor.matmul(
                        pa[:, :384],
                        g2t_sb[:, t, :],
                        w_sb[:, WFC0 + t, ch * 384:(ch + 1) * 384],
                        start=True, stop=True,
                    )
                    copy(a_sb[:, t, ch * 384:(ch + 1) * 384], pa[:, :384])

            # ---- M[d, e] = sum_kf wq[kf, d] A[kf, e] ----
            for dtile in range(DT):
                for ch in range(2):
                    pm = psp.tile([P, 512], F32, tag="ps", name="pm")
                    for kft in range(DT):
                        nc.tensor.matmul(
                            pm[:, :384],
                            w_sb[:, WQ0 + kft, dtile * P:(dtile + 1) * P],
                            a_sb[:, kft, ch * 384:(ch + 1) * 384],
                            start=(kft == 0), stop=(kft == DT - 1),
                        )
                    copy(m_sb[:, dtile, ch * 384:(ch + 1) * 384], pm[:, :384])

            # ---- outT[e, n] = sum_d M[d, e] xT[d, n] + b[e] ----
            # bias scalar operand for tensor_scalar_add must be f32
            nc.scalar.copy(bias_sb[:], w_sb[:, IDB, P:P + DT])
            for et in range(DT):
                ot = outsp.tile([P, N], F16, tag="ot", name="ot")
                bias_ap = bias_sb[:, et:et + 1]
                # last row block ends with two narrow 256-col units so the
                # final matmul->bias->store chain is as short as possible
                chunks = ([(0, 512), (512, 512)] if et < DT - 1
                          else [(0, 512), (512, 256), (768, 256)])
                for c0, w in chunks:
                    po = psp.tile([P, 512], F32, tag="ps", name="po")
                    for dt in range(DT):
                        nc.tensor.matmul(
                            po[:, :w],
                            m_sb[:, dt, et * P:(et + 1) * P],
                            xT_sb[:, dt, c0:c0 + w],
                            start=(dt == 0), stop=(dt == DT - 1),
                        )
                    dst = ot[:, c0:c0 + w]
                    nc.vector.tensor_scalar_add(dst, po[:, :w], bias_ap)
                    if et == DT - 1:
                        # store each piece as soon as its bias-add lands
                        nc.scalar.dma_start(outT_r[:, et, c0:c0 + w], dst)
                if et < DT - 1:
                    # store via the Activation-engine HWDGE (its own
                    # descriptor generator; sync's is busy with loads)
                    nc.scalar.dma_start(outT_r[:, et, :], ot[:])

    nc.compile()
    return nc


_NC_CACHE = None
LAST_EXEC_NS = None
LAST_RES = None


def kernel(x, w_qkv, w_fc, b_fc, _trace=False):
    global _NC_CACHE, LAST_EXEC_NS, LAST_RES
    x = np.asarray(x, dtype=np.float32)
    w_qkv = np.asarray(w_qkv, dtype=np.float32)
    w_fc = np.asarray(w_fc, dtype=np.float32)
    b_fc = np.asarray(b_fc, dtype=np.float32)

    if _NC_CACHE is None:
        _NC_CACHE = _build_program()
    nc = _NC_CACHE

    f16 = np.float16
    wcat = np.zeros((WCAT * P, D), dtype=f16)
    wcat[WV0 * P:(WV0 + 6) * P] = w_qkv[2 * D:].T.astype(f16)          # wvT
    wcat[WK0 * P:(WK0 + 6) * P] = (SCALE * w_qkv[D:2 * D]).T.astype(f16)
    wcat[WFC0 * P:(WFC0 + 6) * P] = w_fc.T.astype(f16)                 # wfcT
    wcat[WQ0 * P:(WQ0 + 6) * P] = w_qkv[:D].astype(f16)                # wqN
    idb = wcat[IDB * P:(IDB + 1) * P]
    idb[:, 0:P] = np.eye(P, dtype=f16)
    idb[:, P:P + DT] = b_fc.astype(f16).reshape(DT, P).T               # bias

    in_maps = []
    for b in range(B):
        in_maps.append({
            "xN": x[b].astype(f16),
            "xT": np.ascontiguousarray(x[b].T).astype(f16),
            "wcat": wcat,
        })

    res = bass_utils.run_bass_kernel_spmd(
        nc, in_maps, core_ids=list(range(B)), trace=_trace
    )
    LAST_EXEC_NS = res.exec_time_ns
    LAST_RES = res
    out = np.stack([res.results[b]["outT"].T.astype(np.float32)
                    for b in range(B)])
    return np.ascontiguousarray(out)
